# revision 35
# baseline (speedup 1.0000x reference)
"""Trainium2 Bass kernel for the ASAP dual-branch GNN (GraphConv mean-aggr).

Strategy (data-parallel over graphs, 32 graphs per NeuronCore):
  * Host folds each graph's edges into a dense normalized adjacency
    An[src, dst] = count/max(deg_dst,1), shipped in fp8e4 (values in [0,1])
    so GraphConv becomes dense matmuls: h = relu(wrel^T (x An) + wroot^T x + b).
  * Work is software-pipelined at (pair, branch)-unit granularity with
    lagged stages so the PE never waits on ACT/DVE ops:
      iter u: yr(u)+root-hp(u) | agg-hp(u-1) | zr(u-2)+root-gp(u-2) | agg-gp(u-3)
    Engine ops issued right after their producers: cast1/relu1/relu2 on ACT,
    cast2/pool on DVE. PSUM: 8 banks = {yr, zr, hp, gp} x 2-buf rotation.
  * Layer-2 aggregation runs fp8 DoubleRow (zs cast to fp8e4, both src-half
    k-tiles packed per matmul = 2x PE rate); layer-1 agg keeps bf16 ys against
    the same fp8 An (mixed-operand matmul) to stay within the accuracy gate.
  * Biases applied via ACT per-partition bias APs in the relus. A single ACT
    table set covering {relu, copy, exp, ln} avoids mid-kernel table loads.
  * Transpose-free MLP head (graph index kept on the free axis) + log_softmax
    per-core in f32; no collectives.
Host side only does sharding/layout: adjacency histogram, transposes,
dtype casts, and folding the mean-pool 1/200 into lin1_w.
"""

import sys

import numpy as np

if "/opt/trn_rl_repo" not in sys.path:
    sys.path.insert(0, "/opt/trn_rl_repo")

B, N, EPG = 256, 200, 3200
F, H, C = 200, 128, 2
NCORES = 8
GPC = B // NCORES  # graphs per core
NPAIR = GPC // 2
NH = 100  # F-half / src-half width

_CACHE = {}


def _f32(x):
    return np.ascontiguousarray(x, dtype=np.float32)


def _build(gpc=GPC):
    import concourse.bass as bass  # noqa: F401
    import concourse.tile as tile
    from concourse import bacc, mybir

    dt = mybir.dt
    AF = mybir.ActivationFunctionType
    OP = mybir.AluOpType
    assert gpc % 2 == 0
    npair = gpc // 2
    nunit = npair * 2  # (pair, branch) units

    # Prefer the combined {relu, copy, exp, ln} activation table set so the
    # whole kernel needs a single ACT table load (saves ~2.6us of serial
    # table loads in the log_softmax tail).
    from concourse.hw_specs import get_activation_tables

    nc = bacc.Bacc("TRN2", target_bir_lowering=False, debug=False)
    # Make every activation resolve to the one set that contains all of
    # {relu, copy, exp, ln} so the kernel needs a single table load. Set
    # membership is mutated in place; dict order (= act_func_set_id) is
    # preserved so the physical tables walrus loads stay correct.
    tabs = get_activation_tables(nc.m.arch)
    if "natural_log_exp_and_others" in tabs:
        for k, v in tabs.items():
            if k != "natural_log_exp_and_others":
                v.clear()

    # ---- DRAM I/O ----
    # x: [pair, Fpart(100), Fhalf, branch, graph, node]
    xt_d = nc.dram_tensor("xt", [npair, NH, 2, 2, 2, N], dt.bfloat16, kind="ExternalInput").ap()
    # An: [pair, srcpart(100), srchalf, branch, graph, dst]  (fp8: values in [0,1])
    an_d = nc.dram_tensor("an", [npair, NH, 2, 2, 2, N], dt.float8e4, kind="ExternalInput").ap()
    w1_d = nc.dram_tensor("w1", [NH, 2, 2, 2, H], dt.bfloat16, kind="ExternalInput").ap()  # [NH, k, fh, b, H]
    w2_d = nc.dram_tensor("w2", [H, 2, 2, H], dt.bfloat16, kind="ExternalInput").ap()
    br_d = nc.dram_tensor("brl", [H, 2, 2, 1], dt.float32, kind="ExternalInput").ap()
    # all MLP constants packed into one f32 blob: [l1w(512) | l2w(64) | l3w(2) |
    #  l1b col(1) | l2b col(1) | l3b row(2) | ones row(32)]
    BW = 512 + 64 + 2 + 1 + 1 + 2 + 32
    blob_d = nc.dram_tensor("blob", [128, BW], dt.float32, kind="ExternalInput").ap()
    out_d = nc.dram_tensor("out", [gpc, C], dt.float32, kind="ExternalOutput").ap()

    with tile.TileContext(nc) as tc:
        with (
            tc.tile_pool(name="cpool", bufs=1) as cpool,
            tc.tile_pool(name="xpool", bufs=6) as xpool,
            tc.tile_pool(name="apool", bufs=6) as apool,
            tc.tile_pool(name="ypool", bufs=2) as ypool,   # ys2/zs2 SBUF bf16
            tc.tile_pool(name="hpool", bufs=5) as hpool,   # h1/g1 merged per unit
            tc.tile_pool(name="rpool", bufs=4) as rpool,
            tc.tile_pool(name="psYZ", bufs=3, space="PSUM") as psYZ,  # yr+zr shared
            tc.tile_pool(name="psH", bufs=3, space="PSUM") as psH,    # hp
            tc.tile_pool(name="psG", bufs=2, space="PSUM") as psG,    # gp
        ):
            # ---- conv weights (sync queue, first so conv starts ASAP) ----
            w1 = cpool.tile([NH, 2, 2, 2, H], dt.bfloat16)  # [NH, k, fh, b, H]
            nc.sync.dma_start(out=w1[:, 0], in_=w1_d[:, 0])  # rel weights first
            brl = cpool.tile([H, 2, 2, 1], dt.float32)
            nc.sync.dma_start(out=brl[:], in_=br_d[:])
            nc.sync.dma_start(out=w1[:, 1], in_=w1_d[:, 1])  # root weights
            w2 = cpool.tile([H, 2, 2, H], dt.bfloat16)
            # ---- MLP consts: one blob DMA on the (idle) vector queue ----
            blob = cpool.tile([128, BW], dt.float32)
            nc.scalar.dma_start(out=blob[:], in_=blob_d[:])

            # pooled readouts: [H, branch, layer, graph] f32
            pooled = cpool.tile([H, 2, 2, gpc], dt.float32, name="pooled")

            # per-pair input tiles and per-unit psum/sbuf tiles, tracked by index
            xts, ants = {}, {}
            yr_t, zr_t, hp_t, gp_t, ys_t, zs_t, hg_t = {}, {}, {}, {}, {}, {}, {}

            def dma_x(q, split=False):
                xts[q] = xpool.tile([NH, 2, 2, 2, N], dt.bfloat16, tag="xt", name=f"xt{q}")
                if split:
                    for b in range(2):
                        nc.gpsimd.dma_start(out=xts[q][:, :, b, :, :], in_=xt_d[q, :, :, b, :, :])
                else:
                    nc.gpsimd.dma_start(out=xts[q][:], in_=xt_d[q])

            def dma_an(q, split=False):
                # even pairs on the sync queue, odd pairs on gpsimd: halves
                # each queue's issue backlog so An never arrives late
                eng = nc.sync if q % 2 == 0 else nc.gpsimd
                ants[q] = apool.tile([NH, 2, 2, 2, N], dt.float8e4, tag="an", name=f"an{q}")
                if split:
                    for b in range(2):
                        eng.dma_start(out=ants[q][:, :, b, :, :], in_=an_d[q, :, :, b, :, :])
                else:
                    eng.dma_start(out=ants[q][:], in_=an_d[q])

            # prefetch first four pairs
            dma_x(0, split=True)
            dma_an(0, split=True)
            dma_an(1)
            dma_x(1)
            dma_x(2)
            nc.sync.dma_start(out=w2[:], in_=w2_d[:])
            dma_an(2)
            dma_an(3)
            dma_x(3)

            def stage_yr(u):
                # ys = x @ wrel for this (pair, branch); also root-hp start
                p, b = u // 2, u % 2
                xt = xts[p]
                yr = psYZ.tile([128, 2, 2, H], dt.float32, tag="yz", name=f"yr{u}")
                yr_t[u] = yr
                for g in range(2):
                    for m in range(2):
                        for fh in range(2):
                            nc.tensor.matmul(
                                yr[0:NH, g, m, :],
                                lhsT=xt[:, fh, b, g, m * NH:(m + 1) * NH],
                                rhs=w1[:, 0, fh, b, :],
                                start=(fh == 0), stop=(fh == 1),
                            )
                hp = psH.tile([128, 2, N], dt.float32, tag="hp", name=f"hp{u}")
                hp_t[u] = hp
                for fh in range(2):
                    nc.tensor.matmul(
                        hp[:, :, :], lhsT=w1[:, 1, fh, b, :], rhs=xt[:, fh, b, :, :],
                        start=(fh == 0), stop=False,
                    )
                # cast1 on ACT: yr psum -> ys2 bf16
                ys = ypool.tile([128, 2, 2, H], dt.bfloat16, tag="ys", name=f"ys{u}")
                ys_t[u] = ys
                nc.scalar.copy(out=ys[:], in_=yr[:])

            def stage_agg_hp(u):
                p, b = u // 2, u % 2
                an, ys, hp = ants[p], ys_t[u], hp_t[u]
                for g in range(2):
                    for m in range(2):
                        nc.tensor.matmul(
                            hp[:, g, :], lhsT=ys[0:NH, g, m, :], rhs=an[:, m, b, g, :],
                            start=False, stop=(g == 1 and m == 1),
                        )
                # relu1 on ACT: hp -> hg[:,0] bf16 (+bias)
                hg = hpool.tile([H, 2, 2, N], dt.bfloat16, tag="hg", name=f"hg{u}")
                hg_t[u] = hg
                nc.scalar.activation(hg[:, 0, :, :], hp[:, :, :], AF.Relu, bias=brl[:, b, 0, :], scale=1.0)

            def stage_zr(u):
                p, b = u // 2, u % 2
                hg = hg_t[u]
                zr = psYZ.tile([128, 2, 2, H], dt.float32, tag="yz", name=f"zr{u}")
                zr_t[u] = zr
                for g in range(2):
                    for m in range(2):
                        nc.tensor.matmul(
                            zr[0:NH, g, m, :],
                            lhsT=hg[:, 0, g, m * NH:(m + 1) * NH],
                            rhs=w2[:, b, 0, :],
                            start=True, stop=True,
                        )
                gp = psG.tile([128, 2, N], dt.float32, tag="gp", name=f"gp{u}")
                gp_t[u] = gp
                nc.tensor.matmul(
                    gp[:, :, :], lhsT=w2[:, b, 1, :], rhs=hg[:, 0, :, :],
                    start=True, stop=False,
                )
                # cast2 on DVE; ACT for the last units, where DVE's serial
                # cast2+pool backlog gates the pipeline flush
                zs = ypool.tile([128, 2, 2, H], dt.float8e4, tag="zs", name=f"zs{u}")
                zs_t[u] = zs
                if u >= nunit - 4:
                    nc.scalar.copy(out=zs[:], in_=zr[:])
                else:
                    nc.vector.tensor_copy(out=zs[:], in_=zr[:])

            def stage_agg_gp(u):
                p, b = u // 2, u % 2
                an, zs, gp = ants[p], zs_t[u], gp_t[u]
                for g in range(2):
                    nc.tensor.matmul(
                        gp[:, g, :], lhsT=zs[0:NH, g, :, :], rhs=an[:, :, b, g, :],
                        start=False, stop=(g == 1),
                        perf_mode=mybir.MatmulPerfMode.DoubleRow,
                    )
                hg = hg_t[u]
                nc.scalar.activation(hg[:, 1, :, :], gp[:, :, :], AF.Relu, bias=brl[:, b, 1, :], scale=1.0)
                nc.vector.tensor_reduce(
                    out=pooled[:, b, :, 2 * p:2 * p + 2], in_=hg[:, :, :, :],
                    axis=mybir.AxisListType.X, op=OP.add,
                )

            # ---- main software-pipelined loop over units ----
            for u in range(nunit + 3):
                if u < nunit:
                    if u % 2 == 0:
                        q = u // 2 + 4
                        if q < npair:
                            dma_x(q)
                            dma_an(q)
                    stage_yr(u)
                if 1 <= u + 0 and u - 1 >= 0 and u - 1 < nunit:
                    stage_agg_hp(u - 1)
                if u - 2 >= 0 and u - 2 < nunit:
                    stage_zr(u - 2)
                if u - 3 >= 0 and u - 3 < nunit:
                    stage_agg_gp(u - 3)

            # ---- MLP head (f32, graph kept on the free axis; no transposes) ----
            l1w_s = blob[:, 0:512]          # [128, 4*H] -> slices [:, k*H:(k+1)*H]
            l2w_s = blob[:, 512:576]        # [128, 64]
            l3w_s = blob[0:H // 2, 576:578]  # [64, 2]
            l1b_s = blob[:, 578:579]        # [128, 1] col
            l2b_s = blob[0:H // 2, 579:580]  # [64, 1] col
            l3b_s = blob[0:1, 580:582]      # [1, 2] row
            ones_s = blob[0:1, 582:582 + gpc]  # [1, gpc] row of ones

            # warm exp/ln ACT tables while PE runs the head matmuls
            warm = rpool.tile([1, 1], dt.float32, tag="warm")
            nc.scalar.activation(warm[:], ones_s[:, 0:1], AF.Exp, bias=0.0, scale=1.0)
            nc.scalar.activation(warm[:], warm[:], AF.Ln, bias=0.0, scale=1.0)

            # z1T [H, g] = sum_k l1w_k^T @ pooled_k  (+bias via ACT)
            t1 = psYZ.tile([128, 2, 2, H], dt.float32, tag="yz", name="headA")
            z1p = t1[:, 0, 0, 0:gpc]
            order = [pooled[:, 0, 0, :], pooled[:, 0, 1, :], pooled[:, 1, 0, :], pooled[:, 1, 1, :]]
            for k in range(4):
                nc.tensor.matmul(z1p, lhsT=blob[:, k * H:(k + 1) * H], rhs=order[k],
                                 start=(k == 0), stop=(k == 3))
            z1s = rpool.tile([H, gpc], dt.float32, tag="z1s")
            nc.scalar.activation(z1s[:], z1p, AF.Relu, bias=l1b_s, scale=1.0)

            # z2T [64, g] = l2w^T @ z1T (+bias)
            t2 = psYZ.tile([128, 2, 2, H], dt.float32, tag="yz", name="headB")
            z2p = t2[0:H // 2, 0, 0, 0:gpc]
            nc.tensor.matmul(z2p, lhsT=l2w_s, rhs=z1s[:], start=True, stop=True)
            z2s = rpool.tile([H // 2, gpc], dt.float32, tag="z2s")
            nc.scalar.activation(z2s[:], z2p, AF.Relu, bias=l2b_s, scale=1.0)

            # z3 [g, 2] = z2T^T @ l3w + l3b
            t3 = psH.tile([128, 2, N], dt.float32, tag="hp", name="headC")
            z3p = t3[0:gpc, 0, 0:C]
            nc.tensor.matmul(z3p, lhsT=z2s[:], rhs=l3w_s, start=True, stop=False)
            nc.tensor.matmul(z3p, lhsT=ones_s, rhs=l3b_s, start=False, stop=True)

            m = rpool.tile([gpc, 1], dt.float32, tag="lsm")
            nc.vector.tensor_reduce(out=m[:], in_=z3p, axis=mybir.AxisListType.X, op=OP.max)
            negm = rpool.tile([gpc, 1], dt.float32, tag="lsnm")
            nc.vector.tensor_scalar(negm[:], m[:], -1.0, None, OP.mult)
            esc = rpool.tile([gpc, C], dt.float32, tag="lse")
            sume = rpool.tile([gpc, 1], dt.float32, tag="lssum")
            nc.scalar.activation(esc[:], z3p, AF.Exp, bias=negm[:], scale=1.0, accum_out=sume[:])
            lse = rpool.tile([gpc, 1], dt.float32, tag="lsl")
            nc.scalar.activation(lse[:], sume[:], AF.Ln, bias=0.0, scale=1.0)
            outv = rpool.tile([gpc, C], dt.float32, tag="outv")
            nc.vector.tensor_scalar(outv[:], z3p, negm[:], lse[:], OP.add, OP.subtract)
            nc.sync.dma_start(out=out_d[:], in_=outv[:])

    nc.compile()
    return nc


def _prep_inputs(sc_x, fc_x, sc_edge_index, fc_edge_index,
                 sc1_wrel, sc1_brel, sc1_wroot, sc2_wrel, sc2_brel, sc2_wroot,
                 fc1_wrel, fc1_brel, fc1_wroot, fc2_wrel, fc2_brel, fc2_wroot,
                 lin1_w, lin1_b, lin2_w, lin2_b, lin3_w, lin3_b, batch=None):
    import ml_dtypes

    bf = ml_dtypes.bfloat16

    def prep_A(ei):
        # dense normalized adjacency: An[g, src, dst] = count / max(deg_dst, 1)
        ei = np.asarray(ei).astype(np.int64)
        gid = np.arange(B * EPG, dtype=np.int64) // EPG
        src = ei[0] - gid * N
        dst = ei[1] - gid * N
        flat = (gid * N + src) * N + dst
        cnt = np.bincount(flat, minlength=B * N * N).astype(np.float32).reshape(B, N, N)
        deg = cnt.sum(axis=1)  # in-degree per dst
        return cnt / np.maximum(deg, 1.0)[:, None, :]

    # x: [pair, Fpart(100), Fhalf, branch, graph, node]
    x = np.stack([np.asarray(sc_x, np.float32), np.asarray(fc_x, np.float32)])  # [2b, B*N, F]
    x = x.reshape(2, B // 2, 2, N, F).transpose(1, 4, 0, 2, 3)  # [pair, F, b, g, n]
    x = x.reshape(B // 2, 2, NH, 2, 2, N).transpose(0, 2, 1, 3, 4, 5)  # [pair, 100, fh, b, g, n]
    xt = np.ascontiguousarray(x).astype(bf)

    # An: [pair, srcpart(100), srchalf, branch, graph, dst]
    An = np.stack([prep_A(sc_edge_index), prep_A(fc_edge_index)])  # [2b, B, src, dst]
    An = An.reshape(2, B // 2, 2, 2, NH, N).transpose(1, 4, 3, 0, 2, 5)  # [pair, 100, sh, b, g, d]
    an = np.ascontiguousarray(An).astype(ml_dtypes.float8_e4m3fn)

    # w1: [Fpart(100), Fhalf, branch, rel/root, H]
    w1 = np.stack([
        np.stack([np.asarray(sc1_wrel), np.asarray(sc1_wroot)]),   # [2k, 200, H]
        np.stack([np.asarray(fc1_wrel), np.asarray(fc1_wroot)]),
    ])  # [2b, 2k, 200, H]
    w1 = w1.reshape(2, 2, 2, NH, H).transpose(3, 1, 2, 0, 4)  # [100, k, fh, b, H]
    w1 = np.ascontiguousarray(w1).astype(bf)

    # w2: [H, branch, rel/root, H]
    w2 = np.stack([
        np.stack([np.asarray(sc2_wrel), np.asarray(sc2_wroot)]),
        np.stack([np.asarray(fc2_wrel), np.asarray(fc2_wroot)]),
    ]).transpose(2, 0, 1, 3)  # [H, b, k, H]
    w2 = np.ascontiguousarray(w2).astype(bf)

    brl = np.stack([
        np.stack([np.asarray(sc1_brel), np.asarray(sc2_brel)]),
        np.stack([np.asarray(fc1_brel), np.asarray(fc2_brel)]),
    ]).astype(np.float32).transpose(2, 0, 1)[:, :, :, None]  # [H, b, l, 1]

    l1w = np.asarray(lin1_w, np.float32).copy()
    l1w[:256] *= 1.0 / N  # fold mean-pool divisor for the SC branch readouts
    l1w = l1w.reshape(4, 128, H).transpose(1, 0, 2)  # [128, 4, H]

    BW = 512 + 64 + 2 + 1 + 1 + 2 + 32
    blob = np.zeros((128, BW), np.float32)
    blob[:, 0:512] = l1w.reshape(128, 512)
    blob[:, 512:576] = _f32(lin2_w)
    blob[0:H // 2, 576:578] = _f32(lin3_w)
    blob[:, 578] = _f32(lin1_b)
    blob[0:H // 2, 579] = _f32(lin2_b)
    blob[0, 580:582] = _f32(lin3_b)
    blob[0, 582:582 + GPC] = 1.0

    return dict(xt=xt, an=an, w1=w1, w2=w2, brl=_c(brl), blob=blob)


def _c(x):
    return np.ascontiguousarray(x)


def _make_in_maps(full):
    in_maps = []
    for c in range(NCORES):
        ps = slice(c * NPAIR, (c + 1) * NPAIR)
        m = dict(full)
        m["xt"] = np.ascontiguousarray(full["xt"][ps])
        m["an"] = np.ascontiguousarray(full["an"][ps])
        in_maps.append(m)
    return in_maps


def kernel(**inputs):
    from concourse import bass_utils

    if "nc" not in _CACHE:
        _CACHE["nc"] = _build()
    nc = _CACHE["nc"]

    full = _prep_inputs(**inputs)
    in_maps = _make_in_maps(full)
    res = bass_utils.run_bass_kernel_spmd(nc, in_maps, list(range(NCORES)))
    return np.concatenate([res.results[i]["out"] for i in range(NCORES)], axis=0).astype(np.float32)
